# revision 1
# baseline (speedup 1.0000x reference)
"""ChannelAttention (CBAM-style) Trainium2 Bass kernel.

Reference computation (per batch image):
    avg = mean(x, spatial)             # [C]
    mx  = max(x, spatial)              # [C]
    s   = sigmoid(mlp(avg) + mlp(max)) # mlp: relu(p@w1+b1)@w2+b2
    y   = x * s[None, None, :]

Full shapes: x [32, 112, 112, 256] f32, w1 [256, 32], b1 [32], w2 [32, 256],
b2 [256].  Data-parallel over batch: each of the 8 NeuronCores handles 4
images; the tiny MLP weights are replicated.

Per-core layout: x viewed as [4, 12544, 256] rows.  Each image is split
into 128 partitions x 98 spatial rows ("(p t) c" with p=128, t=98), and
processed in 7 chunks of 14 spatial rows so DMA (fully contiguous 14336B
per partition per chunk) overlaps compute:
  - sum-pool: PE matmul with an all-ones stationary column accumulating
    [1, 256] channel sums in PSUM across all 98 row-tiles.
  - max-pool: DVE tensor_tensor(max) accumulation into a [128, 7, 256]
    accumulator, tree-folded to [128, 256], then PE-transposed to
    channel-major and reduce_max'd to [128, 1] per 128-channel block.
    (TensorTensor is DVE-only in practice: ACT rejects it architecturally
    and the Pool/Q7 path is not lowerable by this walrus pipeline.)
  - MLP runs on PE/ACT at [K<=128, N<=256] sizes; sigmoid on ACT; the
    [1, 256] scale row is broadcast to [128, 256] via a K=1 outer-product
    matmul with ones.
  - scale: tensor_tensor(mult) with a free-dim-broadcast AP reading the
    scale straight from PSUM, in place on the resident chunk, then stored
    back (ACT-issued HWDGE ring so loads on the SP ring proceed
    independently).

The kernel is HBM-bound: per core it moves 2 x 51.4MB (one read of x, one
write of y) against ~358GB/s shared HBM, a ~287us roofline.  The CoreSim
cost model puts this schedule at ~249us; DVE carries ~54us/img of 1x-rate
fp32 elementwise work, safely under the ~72us/img of real DMA time.  The
first image's loads and the last image's stores alternate across both HWDGE
rings so pipeline fill and drain overlap on two DMA queues.
"""

import sys

import numpy as np

for _p in ("/opt/trn_rl_repo",):
    if _p not in sys.path:
        sys.path.append(_p)

import concourse.bass as bass
import concourse.tile as tile
from concourse import mybir

B, HW, C = 32, 112 * 112, 256  # 12544 spatial positions per image
HID = 32
N_CORES = 8
IMG_PER_CORE = B // N_CORES  # 4
P = 128
T = HW // P  # 98 spatial rows per partition
CHUNK_T = 14  # rows per chunk
N_CHUNK = T // CHUNK_T  # 7
MAXW = 7  # max-pool accumulator width (rows)
F32 = mybir.dt.float32


def _split_multiwait(nc, max_waits=1):
    """This walrus build rejects >1 sync wait per instruction.  Hoist
    excess waits onto InstNoOp instructions inserted just before, on the
    same engine (same semantics: the sequencer blocks on each in turn)."""
    for f in nc.m.functions:
        for bb in f.blocks:
            new_insts = []
            for ins in bb.instructions:
                si = ins.sync_info
                w = list(si.on_wait) if si and si.on_wait else []
                if len(w) > max_waits:
                    for j, ww in enumerate(w[:-max_waits]):
                        nop = mybir.InstNoOp(
                            name=f"{ins.name}.sw{j}",
                            engine=ins.engine,
                            sync_info=mybir.SyncInfo(on_wait=[ww], on_update=[]),
                        )
                        nc.register_instruction(nop, overwrite=True)
                        new_insts.append(nop)
                    si.on_wait = w[-max_waits:]
                new_insts.append(ins)
            bb.instructions = new_insts


def build_nc(n_img=IMG_PER_CORE, chunk_bufs=12, repeat=1, chunk_t=CHUNK_T,
             store_engine="scalar"):
    n_chunk = T // chunk_t
    assert chunk_t % MAXW == 0 or MAXW % chunk_t == 0
    nc = bass.Bass()
    rows = n_img * HW
    x_d = nc.declare_dram_parameter("x", [rows, C], F32, isOutput=False)
    w1_d = nc.declare_dram_parameter("w1", [C, HID], F32, isOutput=False)
    b1_d = nc.declare_dram_parameter("b1", [HID], F32, isOutput=False)
    w2_d = nc.declare_dram_parameter("w2", [HID, C], F32, isOutput=False)
    b2_d = nc.declare_dram_parameter("b2", [C], F32, isOutput=False)
    id_d = nc.declare_dram_parameter("ident", [P, P], F32, isOutput=False)
    y_d = nc.declare_dram_parameter("y", [rows, C], F32, isOutput=True)

    xv = x_d.rearrange("(i p t) c -> i p t c", i=n_img, p=P)
    yv = y_d.rearrange("(i p t) c -> i p t c", i=n_img, p=P)

    AF = mybir.ActivationFunctionType
    OP = mybir.AluOpType

    with tile.TileContext(nc) as tc:
        with (
            tc.tile_pool(name="singles", bufs=1) as singles,
            tc.tile_pool(name="chunks", bufs=chunk_bufs) as chunks_pool,
            tc.tile_pool(name="maxacc", bufs=2) as maxacc_pool,
            tc.tile_pool(name="small", bufs=3) as small,
            tc.tile_pool(name="ps_sum", bufs=2, space="PSUM") as ps_sum_pool,
            tc.tile_pool(name="ps_small", bufs=2, space="PSUM") as ps_small_pool,
            tc.tile_pool(name="ps_bc", bufs=2, space="PSUM") as ps_bc_pool,
        ):
            # --- constants ---
            w1_sb = singles.tile([P, 2, HID], F32)
            nc.sync.dma_start(out=w1_sb[:], in_=w1_d.rearrange("(b p) h -> p b h", p=P))
            w2_sb = singles.tile([HID, C], F32)
            nc.sync.dma_start(out=w2_sb[:], in_=w2_d[:, :])
            b1_sb = singles.tile([HID, 1], F32)
            nc.sync.dma_start(out=b1_sb[:], in_=b1_d.rearrange("(p o) -> p o", o=1))
            b2x2 = singles.tile([1, C], F32)
            nc.sync.dma_start(out=b2x2[:], in_=b2_d.rearrange("(o c) -> o c", o=1))
            nc.scalar.mul(out=b2x2[:], in_=b2x2[:], mul=2.0)
            ident = singles.tile([P, P], F32)
            nc.sync.dma_start(out=ident[:], in_=id_d[:, :])
            ones_col = singles.tile([P, 1], F32)
            nc.vector.memset(ones_col[:], 1.0)
            ones_row = singles.tile([1, P], F32)
            nc.vector.memset(ones_row[:], 1.0)

            imgs = [i for _ in range(repeat) for i in range(n_img)]
            for imgno, img in enumerate(imgs):
                first_img = imgno == 0
                last_img = imgno == len(imgs) - 1
                psum_sum = ps_sum_pool.tile([1, C], F32)
                w = min(MAXW, chunk_t)
                # All elementwise tensor_tensor work must live on DVE: trn2
                # TensorTensor is DVE/Pool-only, ACT rejects it, and the
                # Pool (Q7 software) path is not lowerable by this walrus
                # pipeline.  DVE totals ~54us/img, still under the ~72us/img
                # real HBM time, so the kernel stays DMA-bound.
                maxacc = maxacc_pool.tile([P, MAXW, C], F32, tag="ma")
                cks = []
                for g in range(n_chunk):
                    ck = chunks_pool.tile([P, chunk_t, C], F32, tag="ck")
                    cks.append(ck)
                    # During pipeline fill the store ring (ACT-issued HWDGE)
                    # is idle, so the first image's loads alternate over both
                    # rings to halve the fill time.
                    load_eng = nc.scalar if (first_img and g % 2 == 1) else nc.sync
                    if g == n_chunk - 1:
                        # split the last chunk's load so its pool work (the
                        # head of the per-image MLP critical chain) starts a
                        # half-chunk earlier
                        half = chunk_t // 2
                        load_eng.dma_start(
                            out=ck[:, 0:half, :],
                            in_=xv[img][:, g * chunk_t : g * chunk_t + half, :],
                        )
                        load_eng.dma_start(
                            out=ck[:, half:chunk_t, :],
                            in_=xv[img][:, g * chunk_t + half : (g + 1) * chunk_t, :],
                        )
                    else:
                        load_eng.dma_start(
                            out=ck[:],
                            in_=xv[img][:, g * chunk_t : (g + 1) * chunk_t, :],
                        )
                    # sum-pool: accumulate channel sums in PSUM
                    for t in range(chunk_t):
                        nc.tensor.matmul(
                            psum_sum[:],
                            lhsT=ones_col[:],
                            rhs=ck[:, t, :],
                            start=(g == 0 and t == 0),
                            stop=(g == n_chunk - 1 and t == chunk_t - 1),
                        )
                    # max-pool accumulate in MAXW-row slices
                    for s in range(chunk_t // w):
                        sl = ck[:, s * w : (s + 1) * w, :]
                        if g == 0 and s == 0:
                            nc.vector.tensor_copy(out=maxacc[:, 0:w, :], in_=sl)
                        else:
                            nc.vector.tensor_tensor(
                                out=maxacc[:, 0:w, :],
                                in0=maxacc[:, 0:w, :],
                                in1=sl,
                                op=OP.max,
                            )

                # --- pooled vectors ---
                # avg: psum row -> sbuf row (scaled by 1/HW)
                avg_row = small.tile([1, C], F32, tag="avg_row")
                nc.scalar.activation(
                    out=avg_row[:], in_=psum_sum[:], func=AF.Copy, scale=1.0 / HW
                )
                # max: fold MAXW -> 1, tree-wise, in place
                m = maxacc
                nc.vector.tensor_tensor(m[:, 0, :], m[:, 0, :], m[:, 1, :], op=OP.max)
                nc.vector.tensor_tensor(m[:, 2, :], m[:, 2, :], m[:, 3, :], op=OP.max)
                nc.vector.tensor_tensor(m[:, 4, :], m[:, 4, :], m[:, 5, :], op=OP.max)
                nc.vector.tensor_tensor(m[:, 0, :], m[:, 0, :], m[:, 2, :], op=OP.max)
                nc.vector.tensor_tensor(m[:, 4, :], m[:, 4, :], m[:, 6, :], op=OP.max)
                nc.vector.tensor_tensor(m[:, 0, :], m[:, 0, :], m[:, 4, :], op=OP.max)

                # pooled columns: [128, 4] = avg_b0, avg_b1, max_b0, max_b1
                pooled = small.tile([P, 4], F32, tag="pooled")
                for blk in range(2):
                    pc = ps_small_pool.tile([P, 1], F32, tag="ps")
                    nc.tensor.matmul(
                        pc[:],
                        lhsT=avg_row[:, blk * P : (blk + 1) * P],
                        rhs=ones_col[0:1, 0:1],
                        start=True,
                        stop=True,
                    )
                    nc.any.tensor_copy(out=pooled[:, blk : blk + 1], in_=pc[:])
                for blk in range(2):
                    pt = ps_small_pool.tile([P, P], F32, tag="ps")
                    nc.tensor.transpose(
                        pt[:], m[:, 0, blk * P : (blk + 1) * P], ident[:]
                    )
                    nc.vector.reduce_max(
                        out=pooled[:, 2 + blk : 3 + blk],
                        in_=pt[:],
                        axis=mybir.AxisListType.X,
                    )

                # --- shared MLP on both pooled vectors ---
                h_sb = small.tile([HID, 2], F32, tag="h")
                for j in range(2):  # 0: avg path, 1: max path
                    ph = ps_small_pool.tile([HID, 1], F32, tag="ps")
                    nc.tensor.matmul(
                        ph[:],
                        lhsT=w1_sb[:, 0, :],
                        rhs=pooled[:, 2 * j : 2 * j + 1],
                        start=True,
                        stop=False,
                    )
                    nc.tensor.matmul(
                        ph[:],
                        lhsT=w1_sb[:, 1, :],
                        rhs=pooled[:, 2 * j + 1 : 2 * j + 2],
                        start=False,
                        stop=True,
                    )
                    nc.scalar.activation(
                        out=h_sb[:, j : j + 1],
                        in_=ph[:],
                        func=AF.Relu,
                        bias=b1_sb[:],
                        scale=1.0,
                    )
                py = ps_small_pool.tile([1, C], F32, tag="ps")
                nc.tensor.matmul(
                    py[:], lhsT=h_sb[:, 0:1], rhs=w2_sb[:], start=True, stop=False
                )
                nc.tensor.matmul(
                    py[:], lhsT=h_sb[:, 1:2], rhs=w2_sb[:], start=False, stop=True
                )
                sig_row = small.tile([1, C], F32, tag="sig")
                nc.vector.tensor_add(out=sig_row[:], in0=py[:], in1=b2x2[:])
                nc.scalar.activation(out=sig_row[:], in_=sig_row[:], func=AF.Sigmoid)

                # broadcast scale row to all 128 partitions (outer product);
                # the scale muls read this PSUM tile directly (saves the
                # SBUF-copy hop on the boundary critical path)
                pbc = ps_bc_pool.tile([P, C], F32)
                nc.tensor.matmul(
                    pbc[:], lhsT=ones_row[:], rhs=sig_row[:], start=True, stop=True
                )
                srep3 = pbc[:, :].rearrange("p (o c) -> p o c", o=1)

                # --- scale in place and store ---
                base_store = {"scalar": nc.scalar, "sync": nc.sync,
                              "gpsimd": nc.gpsimd}[store_engine]
                for g in range(n_chunk):
                    # During pipeline drain the load ring is idle, so the
                    # last image's stores alternate over both rings.
                    store_eng = nc.sync if (last_img and g % 2 == 1) else base_store
                    ck = cks[g]
                    if g == 0:
                        # split the first mul+store so the store ring starts
                        # draining a half-mul earlier after the MLP
                        half = chunk_t // 2
                        for h0, h1 in ((0, half), (half, chunk_t)):
                            v3 = ck[:, h0:h1, :]
                            _, s_b = bass.broadcast_tensor_aps(v3, srep3)
                            nc.vector.tensor_tensor(out=v3, in0=v3, in1=s_b, op=OP.mult)
                            store_eng.dma_start(
                                out=yv[img][:, g * chunk_t + h0 : g * chunk_t + h1, :],
                                in_=ck[:, h0:h1, :],
                            )
                    else:
                        v3 = ck[:, :, :]
                        _, s_b = bass.broadcast_tensor_aps(v3, srep3)
                        nc.vector.tensor_tensor(out=v3, in0=v3, in1=s_b, op=OP.mult)
                        store_eng.dma_start(
                            out=yv[img][:, g * chunk_t : (g + 1) * chunk_t, :], in_=ck[:]
                        )

    _split_multiwait(nc)
    return nc


# ---------------------------------------------------------------------------
# host-side driver
# ---------------------------------------------------------------------------

_CACHED = {}


def _get_nc():
    if "nc" not in _CACHED:
        _CACHED["nc"] = build_nc()
    return _CACHED["nc"]


def kernel(x, w1, b1, w2, b2):
    from concourse.bass_utils import run_bass_kernel_spmd

    x = np.ascontiguousarray(np.asarray(x), dtype=np.float32)
    assert x.shape == (B, 112, 112, C)
    xr = x.reshape(B, HW, C)
    ident = np.eye(P, dtype=np.float32)
    in_maps = []
    for c in range(N_CORES):
        shard = np.ascontiguousarray(
            xr[c * IMG_PER_CORE : (c + 1) * IMG_PER_CORE].reshape(
                IMG_PER_CORE * HW, C
            )
        )
        in_maps.append(
            {
                "x": shard,
                "w1": np.ascontiguousarray(w1, dtype=np.float32),
                "b1": np.ascontiguousarray(b1, dtype=np.float32),
                "w2": np.ascontiguousarray(w2, dtype=np.float32),
                "b2": np.ascontiguousarray(b2, dtype=np.float32),
                "ident": ident,
            }
        )
    nc = _get_nc()
    res = run_bass_kernel_spmd(nc, in_maps, list(range(N_CORES)))
    out = np.empty((B, HW, C), dtype=np.float32)
    for c in range(N_CORES):
        out[c * IMG_PER_CORE : (c + 1) * IMG_PER_CORE] = res.results[c]["y"].reshape(
            IMG_PER_CORE, HW, C
        )
    return out.reshape(B, 112, 112, C)

